# revision 16
# baseline (speedup 1.0000x reference)
"""Trainium2 Bass kernel for int8-valued Conv2d(128->256, 3x3, pad 1) + BN-add +
shift requant + clip + uint8 cast, over x[32,128,56,56].

Strategy: data-parallel over batch across 8 NeuronCores (4 images/core).
Per core, the conv runs as 9 PSUM-accumulated bf16 matmuls (one per 3x3 tap)
with Cin=128 on the partition axis. int8-valued data is exact in bf16, and all
fp32 accumulations stay below 2^24, so the matmul path is integer-exact.
Input is laid out zero-padded to 58x58 in SBUF so each tap's rhs is a plain
column-shifted window. Output rows are produced in chunks of 8 rows
(N = 8*58 = 464 <= 512 fp32 PSUM bank columns).

Requant (matches reference op order): ACT adds per-channel t (fp32, integral,
out int32), DVE arithmetic-shifts right by s (int32->int32; bitwise ops cannot
cast), DVE clamps to [act_min, act_max] (fp32 ALU pair, writes uint8). Every
dtype conversion happens on integral values, so rounding-mode differences
between sim and HW are moot.
"""

import numpy as np
import ml_dtypes
from contextlib import ExitStack

import concourse.bass as bass  # noqa: F401  (registers engine types)
import concourse.mybir as mybir
import concourse.tile as tile
from concourse import bacc
from concourse.bass_utils import run_bass_kernel_spmd

# Problem constants (hardcoded per contract)
N_CORES = 8
B = 32
B_LOC = B // N_CORES          # 4 images per core
P = 128                       # Cin = partition dim
H = W = 56
Hp = Wp = 58                  # padded
IMG = Hp * Wp                 # 3364 padded pixels per image
NB = B_LOC * IMG + 2          # + leading/trailing guard column
COUT = 256
G = COUT // P                 # 2 Cout halves
ROWS_PER_CH = 8
N_CH = H // ROWS_PER_CH       # 7 chunks per image
NFREE = ROWS_PER_CH * Wp      # 464 columns per matmul

_cache = {}


def _build(shift: int):
    """Build + compile the per-core Bass program. Same NEFF on all 8 cores."""
    nc = bacc.Bacc("TRN2", target_bir_lowering=False, debug=False,
                   num_devices=N_CORES)

    xs = nc.dram_tensor("xs", [B_LOC, P, H, W], mybir.dt.int8, kind="ExternalInput")
    wt = nc.dram_tensor("wt", [P, 9 * COUT], mybir.dt.bfloat16, kind="ExternalInput")
    tb = nc.dram_tensor("tb", [P, G], mybir.dt.float32, kind="ExternalInput")
    lo = nc.dram_tensor("lo", [P, G], mybir.dt.float32, kind="ExternalInput")
    hi = nc.dram_tensor("hi", [P, G], mybir.dt.float32, kind="ExternalInput")
    ys = nc.dram_tensor("ys", [B_LOC, COUT, H, W], mybir.dt.uint8, kind="ExternalOutput")

    with tile.TileContext(nc) as tc, ExitStack() as ctx:
        wpool = ctx.enter_context(tc.tile_pool(name="wpool", bufs=1))
        cpool = ctx.enter_context(tc.tile_pool(name="cpool", bufs=1))
        xspool = ctx.enter_context(tc.tile_pool(name="xspool", bufs=3))
        xppool = ctx.enter_context(tc.tile_pool(name="xppool", bufs=1))
        pspool = ctx.enter_context(tc.tile_pool(name="pspool", bufs=6, space="PSUM"))
        i1pool = ctx.enter_context(tc.tile_pool(name="i1pool", bufs=2))
        i2pool = ctx.enter_context(tc.tile_pool(name="i2pool", bufs=2))
        opool = ctx.enter_context(tc.tile_pool(name="opool", bufs=3))

        # g-major weight layout; split DMA so g=0's taps arrive first
        wt_s = wpool.tile([P, 9 * COUT], mybir.dt.bfloat16)
        nc.sync.dma_start(wt_s[:, :9 * P], wt.ap()[:, :9 * P])
        nc.sync.dma_start(wt_s[:, 9 * P:], wt.ap()[:, 9 * P:])
        tb_s = cpool.tile([P, G], mybir.dt.float32)
        nc.sync.dma_start(tb_s[:], tb.ap())
        lo_s = cpool.tile([P, G], mybir.dt.float32)
        nc.sync.dma_start(lo_s[:], lo.ap())
        hi_s = cpool.tile([P, G], mybir.dt.float32)
        nc.sync.dma_start(hi_s[:], hi.ap())

        xpad = xppool.tile([P, NB], mybir.dt.bfloat16)
        nc.vector.memset(xpad[:, 0:1], 0.0)
        nc.vector.memset(xpad[:, NB - 1:NB], 0.0)
        inners = []
        for img in range(B_LOC):
            base = 1 + img * IMG
            # pad row 0 (+ row 1 col 0)
            nc.vector.memset(xpad[:, base:base + Wp + 1], 0.0)
            # pad row Hp-1
            nc.vector.memset(xpad[:, base + (Hp - 1) * Wp: base + IMG], 0.0)
            inner = xpad[:, base + Wp: base + Wp + H * Wp].rearrange(
                "p (h w) -> p h w", w=Wp)
            nc.vector.memset(inner[:, :, Wp - 1:Wp], 0.0)   # col Wp-1, rows 1..H
            nc.vector.memset(inner[:, 1:, 0:1], 0.0)        # col 0, rows 2..H
            inners.append(inner)

        # half-image granularity so the first matmuls can start sooner
        HHALF = H // 2
        for img in range(B_LOC):
            for half in range(2):
                r0 = half * HHALF
                xstage = xspool.tile([P, HHALF * W], mybir.dt.int8)
                nc.sync.dma_start(
                    xstage[:],
                    xs.ap()[img, :, r0:r0 + HHALF, :].rearrange("c h w -> c (h w)"))
                # cast int8 -> bf16 into padded interior (exact: |x| <= 256)
                nc.scalar.copy(inners[img][:, r0:r0 + HHALF, 1:1 + W],
                               xstage[:].rearrange("p (h w) -> p h w", w=W))

        for img in range(B_LOC):
            for g in range(G):
                base = 1 + img * IMG
                for ch in range(N_CH):
                    hp0 = 1 + ch * ROWS_PER_CH
                    psumt = pspool.tile([P, NFREE], mybir.dt.float32)
                    for tap in range(9):
                        dh, dw = tap // 3, tap % 3
                        rs = base + (hp0 + dh - 1) * Wp + (dw - 1)
                        nc.tensor.matmul(
                            psumt[:],
                            lhsT=wt_s[:, (g * 9 + tap) * P:(g * 9 + tap + 1) * P],
                            rhs=xpad[:, rs:rs + NFREE],
                            start=(tap == 0),
                            stop=(tap == 8),
                        )
                    it = i1pool.tile([P, NFREE], mybir.dt.int32)
                    nc.scalar.activation(it[:], psumt[:],
                                         mybir.ActivationFunctionType.Identity,
                                         bias=tb_s[:, g:g + 1], scale=1.0)
                    ct = i2pool.tile([P, NFREE], mybir.dt.int32)
                    nc.vector.tensor_scalar(ct[:], it[:], shift, None,
                                            mybir.AluOpType.arith_shift_right)
                    # clamp + compact away pad columns: strided read, tight write
                    ot = opool.tile([P, ROWS_PER_CH * W], mybir.dt.uint8)
                    csrc = ct[:].rearrange("p (r w) -> p r w", w=Wp)[:, :, 1:1 + W]
                    nc.vector.tensor_scalar(
                        ot[:].rearrange("p (r w) -> p r w", w=W), csrc,
                        lo_s[:, g:g + 1], hi_s[:, g:g + 1],
                        mybir.AluOpType.max, mybir.AluOpType.min)
                    # 448B contiguous per partition on both sides
                    nc.sync.dma_start(
                        ys.ap()[img, g * P:(g + 1) * P,
                                hp0 - 1:hp0 - 1 + ROWS_PER_CH, :]
                        .rearrange("c h w -> c (h w)"),
                        ot[:])

    nc.compile()
    return nc


def _pack_inputs(x, weight, t, n, act_min, act_max):
    x = np.asarray(x)
    weight = np.asarray(weight)
    t = np.asarray(t).reshape(COUT)
    n = np.asarray(n).reshape(COUT)
    act_min = np.asarray(act_min).reshape(COUT)
    act_max = np.asarray(act_max).reshape(COUT)

    assert x.shape == (B, P, H, W) and weight.shape == (COUT, P, 3, 3)
    nval = int(n[0])
    assert np.all(n == nval) and nval <= 0, "non-uniform/positive BN shift unsupported"
    shift = -nval
    assert np.all(act_min >= 0) and np.all(act_max <= 255), \
        "act range must fit uint8 (pure_positive path)"
    # int8 shipping + bf16/fp32 exactness preconditions
    assert x.min() >= -128 and x.max() <= 127
    assert np.abs(weight).max(initial=0) <= 256

    # lhsT pack, g-major: wt[ci, (g*9+tap)*P + co] = weight[g*P+co, ci, kh, kw]
    wr = weight.reshape(G, P, P, 9)            # [g, co, ci, tap]
    wr = wr.transpose(2, 0, 3, 1)              # [ci, g, tap, co]
    wt_np = np.ascontiguousarray(wr.reshape(P, 9 * COUT)).astype(ml_dtypes.bfloat16)

    tb_np = np.ascontiguousarray(t.reshape(G, P).T).astype(np.float32)
    lo_np = np.ascontiguousarray(act_min.reshape(G, P).T).astype(np.float32)
    hi_np = np.ascontiguousarray(act_max.reshape(G, P).T).astype(np.float32)
    return x, wt_np, tb_np, lo_np, hi_np, shift


def kernel(x, weight, t, n, act_min, act_max):
    x, wt_np, tb_np, lo_np, hi_np, shift = _pack_inputs(
        x, weight, t, n, act_min, act_max)

    if shift not in _cache:
        _cache[shift] = _build(shift)
    nc = _cache[shift]

    x8 = x.astype(np.int8)  # exact: setup guarantees int8-valued data
    in_maps = []
    for c in range(N_CORES):
        in_maps.append({
            "xs": np.ascontiguousarray(x8[c * B_LOC:(c + 1) * B_LOC]),
            "wt": wt_np,
            "tb": tb_np,
            "lo": lo_np,
            "hi": hi_np,
        })
    res = run_bass_kernel_spmd(nc, in_maps, core_ids=list(range(N_CORES)))
    out = np.concatenate([res.results[c]["ys"] for c in range(N_CORES)], axis=0)
    return out
